# revision 31
# baseline (speedup 1.0000x reference)
"""DiscreteBipartiteFlow forward on 8 trn2 NeuronCores.

Math: inputs rows are exact one-hots (x0|x1). net = relu(x0@W1+b1)@W2+b2
only depends on i0=argmax(x0), so precompute (on device, per core) the
[V, 2V] table NET = relu(W1+b1)@W2+b2 and its per-row argmaxes
L[i]=argmax(NET[i,:V]), S[i]=argmax(NET[i,V:]). The straight-through
one_hot_argmax is numerically exactly-hard, one_hot_multiply of one-hots
is an index product, one_hot_add an index sum, so
z1 = one_hot((L[i0] + a1*S[i0]) mod V) (0 when S[i0]==0). Out = [x0|z1].

v10 (per core, 1024 rows, 8 rows per partition). Real-HW profile is
descriptor-count-bound on DMA (~200ns/descriptor/queue for 2-8KB) and
DVE/latency-bound on the dependent chain:
 - 3 load DMAs / 384 descriptors, fused by when they're needed:
   A = w1t|b1|w2-scale-half (4.1KB/partition), B = w2-loc-half (+b2
   when nonzero), C = input (8KB). w2 halves host-permuted to be
   contiguous per partition. iota/ident/ones are generated on the Pool
   engine (iota / affine_select / memset) -- no const DMA.
 - scale head computed FIRST on PE so its argmax+pack chain overlaps
   the loc-head matmuls; b2 matmuls skipped when b2 == 0 (host-checked;
   spec fill is zeros).
 - pack row pk[i] = 256*L[i] + 32768*S[i] + 2^22*[S[i]==0] replicated
   to all partitions via TWO PSUM-accumulated diag matmuls
   (crep = ones @ (ident*packS) + ones @ (ident*packL)); the lookup
   mult reads crep straight from PSUM (no copy). The loc-side pack
   arithmetic runs on the Activation engine (per-partition scale AP);
   WAW gate copies pin the DVE stream order to S-chain -> a1 mult ->
   loc argmax -> a1 reduce so DVE never stalls on x arrival.
 - data side: one batched mult+reduce of the x1 half vs iota (a1), one
   of the x0 half vs crep; comb = pk[i0] + a1, exact in fp32 (< 2^23).
 - int32 unpack (10 ops), z1 built in place over the x1 half via two
   broadcast is_equal chunks, two fused stores (4KB descriptors).
Data-parallel over 8 cores; weights replicated.
"""

import numpy as np

V = 128
H = 512
N_CORES = 8
P = 128
NJ = 8               # row slots per partition
NCH = 2              # store chunks
CJ = NJ // NCH

KH = H // P
# tensor A field offsets (f32 elements per partition): w1t | b1 | w2scale
A_W1T = 0
A_B1 = KH * V
A_W2S = KH * V + KH
A_W = KH * V + KH + KH * V
# tensor B field offsets: iota | ident | ones | b2 | w2loc
B_B2 = 0


def _tb_layout(use_b2):
    w2l_of = 2 * V if use_b2 else 0
    return w2l_of, w2l_of + KH * V


def build_bass(rows: int, use_b2: bool):
    """Build the single-core Bass program for a [rows, 2V] batch shard."""
    import concourse.bacc as bacc
    import concourse.bass as bass
    import concourse.tile as tile
    from concourse import mybir

    f32 = mybir.dt.float32
    i32 = mybir.dt.int32
    u32 = mybir.dt.uint32
    A = mybir.AluOpType
    AF = mybir.ActivationFunctionType

    assert rows == P * NJ
    B_W2L, B_W = _tb_layout(use_b2)

    nc = bacc.Bacc(None)
    x = nc.declare_dram_parameter("x", [rows, 2 * V], f32, isOutput=False)
    ta = nc.declare_dram_parameter("ta", [P, A_W], f32, isOutput=False)
    tb = nc.declare_dram_parameter("tb", [P, B_W], f32, isOutput=False)
    out = nc.declare_dram_parameter("out", [rows, 2 * V], f32, isOutput=True)

    x_r = x.rearrange("(p j) n -> p j n", j=NJ)
    out_r = out.rearrange("(p j) n -> p j n", j=NJ)

    def bcast_mid(t_ap, reps):
        return bass.AP(
            tensor=t_ap.tensor, offset=t_ap.offset,
            ap=[t_ap.ap[0], [0, reps]] + list(t_ap.ap[1:]),
        )

    def bcast_last(t_ap, reps):
        return bass.AP(
            tensor=t_ap.tensor, offset=t_ap.offset,
            ap=list(t_ap.ap) + [[0, reps]],
        )

    with tile.TileContext(nc) as tc:
        with (
            tc.tile_pool(name="tab", bufs=1) as tab,
            tc.tile_pool(name="loop", bufs=1) as loop,
            tc.tile_pool(name="ps_net", bufs=2, space="PSUM") as ps_net,
            tc.tile_pool(name="ps_row", bufs=1, space="PSUM") as ps_row,
        ):
            # ---- t0: 3 load DMAs on the SP sequencer, in order of need ----
            ta_sb = tab.tile([P, A_W], f32)
            nc.sync.dma_start(out=ta_sb, in_=ta[:, :])
            tb_sb = tab.tile([P, B_W], f32)
            nc.sync.dma_start(out=tb_sb, in_=tb[:, :])
            xt = loop.tile([P, NJ, 2 * V], f32)
            nc.sync.dma_start(out=xt, in_=x_r[:, :, :])

            w1t_sb = ta_sb[:, A_W1T : A_W1T + KH * V].rearrange("p (k i) -> p k i", k=KH)
            b1_sb = ta_sb[:, A_B1 : A_B1 + KH]
            w2s = ta_sb[:, A_W2S : A_W2S + KH * V].rearrange("p (k c) -> p k c", k=KH)
            w2l = tb_sb[:, B_W2L : B_W2L + KH * V].rearrange("p (k c) -> p k c", k=KH)

            # on-device constants (Pool engine, no input deps)
            iota_i = tab.tile([P, V], i32)
            nc.gpsimd.iota(iota_i, pattern=[[1, V]], base=0, channel_multiplier=0)
            iota_f = tab.tile([P, V], f32)
            nc.vector.tensor_copy(iota_f, iota_i)
            ones_pv = tab.tile([P, V], f32)
            nc.gpsimd.memset(ones_pv, 1.0)
            ident = tab.tile([P, V], f32)
            nc.gpsimd.affine_select(
                out=ident, in_=ones_pv, pattern=[[1, V]],
                compare_op=A.is_equal, fill=0.0, base=0, channel_multiplier=-1,
            )

            # ---- table: NET = relu(W1+b1) @ W2 (+ b2), scale head first ----
            hT = tab.tile([P, KH, V], f32)
            for k in range(KH):
                nc.scalar.activation(
                    hT[:, k, :], w1t_sb[:, k, :], AF.Relu,
                    bias=b1_sb[:, k : k + 1], scale=1.0,
                )
            halves = {}
            for head, w2half in ((1, w2s), (0, w2l)):  # 1=scale first, 0=loc
                ps = ps_net.tile([P, V], f32, tag=f"net{head}")
                for k in range(KH):
                    last = k == KH - 1 and not use_b2
                    nc.tensor.matmul(
                        ps, lhsT=hT[:, k, :], rhs=w2half[:, k],
                        start=(k == 0), stop=last,
                    )
                if use_b2:
                    nc.tensor.matmul(
                        ps, lhsT=ones_pv[0:1, :],
                        rhs=tb_sb[0:1, B_B2 + head * V : B_B2 + (head + 1) * V],
                        start=False, stop=True,
                    )
                halves[head] = ps

            crep_ps = ps_row.tile([P, V], f32)

            with tc.high_priority():
                # scale-head argmax + its half of the pack row
                ixS = tab.tile([P, 8], u32)
                m8S = tab.tile([P, 8], f32)
                nc.vector.max(m8S, halves[1])
                nc.vector.max_index(ixS, m8S, halves[1])
                sfT = tab.tile([P, 1], f32)
                nc.vector.tensor_copy(sfT, ixS[:, 0:1])
                zf = tab.tile([P, 1], f32)
                nc.vector.tensor_scalar(out=zf, in0=sfT, scalar1=0.5, scalar2=None, op0=A.is_le)
                qS = tab.tile([P, 1], f32)
                nc.vector.tensor_scalar(out=qS, in0=sfT, scalar1=32768.0, scalar2=None, op0=A.mult)
                qS2 = tab.tile([P, 1], f32)
                nc.vector.tensor_scalar(out=qS2, in0=zf, scalar1=float(1 << 22), scalar2=qS, op0=A.mult, op1=A.add)
                diagS = tab.tile([P, V], f32)
                nc.vector.tensor_scalar(out=diagS, in0=ident, scalar1=qS2, scalar2=None, op0=A.mult)
                nc.tensor.matmul(crep_ps, lhsT=ones_pv, rhs=diagS, start=True, stop=False)

            # ---- a1 mult (x arrives early now): after the S-chain, before
            # the loc argmax; gates pin the DVE order S-chain -> a1 mult ->
            # maxL/findL -> a1 reduce so nothing stalls on x or NET-loc.
            a1f = tab.tile([P, NJ], f32)
            a1s = loop.tile([P, NJ, V], f32, tag="a1scr")
            nc.vector.tensor_copy(a1s[0:1, 0, 0:1], diagS[0:1, 0:1])
            nc.vector.tensor_mul(a1s, xt[:, :, V :], bcast_mid(iota_f, NJ))

            # loc-head argmax; pack arithmetic on Act (overlaps DVE)
            ixL = tab.tile([P, 8], u32)
            m8L = tab.tile([P, 8], f32)
            nc.vector.tensor_copy(m8L[0:1, 0:1], a1s[0:1, 0, 0:1])
            nc.vector.max(m8L, halves[0])
            nc.vector.max_index(ixL, m8L, halves[0])
            qL256 = tab.tile([P, 1], f32)
            nc.scalar.activation(qL256, ixL[:, 0:1], AF.Copy, scale=256.0)
            diagL = tab.tile([P, V], f32)
            nc.scalar.activation(diagL, ident, AF.Copy, scale=qL256)
            nc.tensor.matmul(crep_ps, lhsT=ones_pv, rhs=diagL, start=False, stop=True)

            nc.vector.tensor_copy(a1f[0:1, 0:1], ixL[0:1, 0:1])
            nc.vector.reduce_sum(a1f, a1s, axis=mybir.AxisListType.X)

            # ---- comb = pk[i0] + a1 per row (batched mult + reduce) ----
            comb_f = tab.tile([P, NJ], f32)
            lks = loop.tile([P, NJ, V], f32, tag="lkscr")
            nc.vector.tensor_mul(lks, xt[:, :, 0:V], bcast_mid(crep_ps, NJ))
            nc.vector.reduce_sum(comb_f, lks, axis=mybir.AxisListType.X)
            nc.vector.tensor_add(comb_f, comb_f, a1f)

            # ---- unpack: c = (S*a1 + L) & 127 | 256*[S==0] ----
            combi = tab.tile([P, NJ], i32)
            nc.vector.tensor_copy(combi, comb_f)
            a1i = tab.tile([P, NJ], i32)
            nc.vector.tensor_scalar(out=a1i, in0=combi, scalar1=V - 1, scalar2=None, op0=A.bitwise_and)
            li = tab.tile([P, NJ], i32)
            nc.vector.tensor_scalar(out=li, in0=combi, scalar1=8, scalar2=V - 1, op0=A.arith_shift_right, op1=A.bitwise_and)
            si = tab.tile([P, NJ], i32)
            nc.vector.tensor_scalar(out=si, in0=combi, scalar1=15, scalar2=V - 1, op0=A.arith_shift_right, op1=A.bitwise_and)
            kill = tab.tile([P, NJ], i32)
            nc.vector.tensor_scalar(out=kill, in0=combi, scalar1=14, scalar2=2 * V, op0=A.arith_shift_right, op1=A.bitwise_and)
            ti = tab.tile([P, NJ], i32)
            nc.vector.tensor_mul(ti, si, a1i)
            nc.vector.tensor_add(ti, ti, li)
            ci = tab.tile([P, NJ], i32)
            nc.vector.tensor_scalar(out=ci, in0=ti, scalar1=V - 1, scalar2=None, op0=A.bitwise_and)
            nc.vector.tensor_tensor(out=ci, in0=ci, in1=kill, op=A.bitwise_or)

            # ---- z1 in place over the x1 half + fused store per chunk ----
            for ch in range(NCH):
                js = ch * CJ
                nc.vector.tensor_tensor(
                    out=xt[:, js : js + CJ, V :],
                    in0=bcast_mid(iota_i, CJ),
                    in1=bcast_last(ci[:, js : js + CJ], V),
                    op=A.is_equal,
                )
                nc.sync.dma_start(
                    out=out_r[:, js : js + CJ, :], in_=xt[:, js : js + CJ, :]
                )

    nc.finalize()
    return nc


def _host_w2_halves(W2):
    # w2half[p, k*V + c] = W2[k*P + p, half*V + c]; returns (scale, loc)
    w4 = W2.reshape(KH, P, 2, V).transpose(1, 2, 0, 3)  # [p, half, k, c]
    loc = w4[:, 0].reshape(P, KH * V)
    scale = w4[:, 1].reshape(P, KH * V)
    return scale, loc


def _host_ta(W1, b1, W2) -> np.ndarray:
    ta = np.zeros((P, A_W), np.float32)
    # w1t[p, k*V + i] = W1[i, k*P + p] -- pure layout marshalling
    ta[:, A_W1T : A_W1T + KH * V] = (
        W1.T.reshape(KH, P, V).transpose(1, 0, 2).reshape(P, KH * V)
    )
    ta[:, A_B1 : A_B1 + KH] = b1.reshape(KH, P).T
    ta[:, A_W2S : A_W2S + KH * V] = _host_w2_halves(W2)[0]
    return np.ascontiguousarray(ta)


def _host_tb(W2, b2, use_b2) -> np.ndarray:
    B_W2L, B_W = _tb_layout(use_b2)
    tb = np.zeros((P, B_W), np.float32)
    if use_b2:
        tb[:, B_B2 : B_B2 + 2 * V] = b2.reshape(1, 2 * V)
    tb[:, B_W2L : B_W2L + KH * V] = _host_w2_halves(W2)[1]
    return np.ascontiguousarray(tb)


# Test-harness hooks: extra kwargs for run_bass_kernel_spmd (e.g. trace=True)
# and the last BassKernelResults for profiling. Unused when graded.
RUN_KWARGS: dict = {}
LAST_RESULTS = None


def kernel(**inputs) -> np.ndarray:
    global LAST_RESULTS
    from concourse.bass_utils import run_bass_kernel_spmd

    x = np.ascontiguousarray(np.asarray(inputs["inputs"], dtype=np.float32))
    W1 = np.asarray(inputs["W1"], dtype=np.float32)
    b1 = np.asarray(inputs["b1"], dtype=np.float32)
    W2 = np.asarray(inputs["W2"], dtype=np.float32)
    b2 = np.asarray(inputs["b2"], dtype=np.float32)
    use_b2 = bool(np.any(b2 != 0.0))

    tan = _host_ta(W1, b1, W2)
    tbn = _host_tb(W2, b2, use_b2)

    B = x.shape[0]
    rows = B // N_CORES
    nc = build_bass(rows, use_b2)

    shards = np.split(x, N_CORES, axis=0)
    in_maps = [{"x": s, "ta": tan, "tb": tbn} for s in shards]
    res = run_bass_kernel_spmd(nc, in_maps, list(range(N_CORES)), **RUN_KWARGS)
    LAST_RESULTS = res
    return np.concatenate([r["out"] for r in res.results], axis=0)


# revision 32
# speedup vs baseline: 1.0192x; 1.0192x over previous
"""DiscreteBipartiteFlow forward on 8 trn2 NeuronCores.

Math: inputs rows are exact one-hots (x0|x1). net = relu(x0@W1+b1)@W2+b2
only depends on i0=argmax(x0), so precompute (on device, per core) the
[V, 2V] table NET = relu(W1+b1)@W2+b2 and its per-row argmaxes
L[i]=argmax(NET[i,:V]), S[i]=argmax(NET[i,V:]). The straight-through
one_hot_argmax is numerically exactly-hard, one_hot_multiply of one-hots
is an index product, one_hot_add an index sum, so
z1 = one_hot((L[i0] + a1*S[i0]) mod V) (0 when S[i0]==0). Out = [x0|z1].

v10 (per core, 1024 rows, 8 rows per partition). Real-HW profile is
descriptor-count-bound on DMA (~200ns/descriptor/queue for 2-8KB) and
DVE/latency-bound on the dependent chain:
 - 3 load DMAs / 384 descriptors, fused by when they're needed:
   A = w1t|b1|w2-scale-half (4.1KB/partition), B = w2-loc-half (+b2
   when nonzero), C = input (8KB). w2 halves host-permuted to be
   contiguous per partition. iota/ident/ones are generated on the Pool
   engine (iota / affine_select / memset) -- no const DMA.
 - scale head computed FIRST on PE so its argmax+pack chain overlaps
   the loc-head matmuls; b2 matmuls skipped when b2 == 0 (host-checked;
   spec fill is zeros).
 - pack row pk[i] = 256*L[i] + 32768*S[i] + 2^22*[S[i]==0] replicated
   to all partitions via TWO PSUM-accumulated diag matmuls
   (crep = ones @ (ident*packS) + ones @ (ident*packL)); the lookup
   mult reads crep straight from PSUM (no copy). The loc-side pack
   arithmetic runs on the Activation engine (per-partition scale AP);
   WAW gate copies pin the DVE stream order to S-chain -> a1 mult ->
   loc argmax -> a1 reduce so DVE never stalls on x arrival.
 - data side: one batched mult+reduce of the x1 half vs iota (a1), one
   of the x0 half vs crep; comb = pk[i0] + a1, exact in fp32 (< 2^23).
 - int32 unpack (10 ops), z1 built in place over the x1 half via two
   broadcast is_equal chunks, two fused stores (4KB descriptors).
Data-parallel over 8 cores; weights replicated.
"""

import numpy as np

V = 128
H = 512
N_CORES = 8
P = 128
NJ = 8               # row slots per partition
NCH = 2              # store chunks
CJ = NJ // NCH

KH = H // P
# tensor A field offsets (f32 elements per partition): w1t | b1 | w2scale
A_W1T = 0
A_B1 = KH * V
A_W2S = KH * V + KH
A_W = KH * V + KH + KH * V
# tensor B field offsets: iota | ident | ones | b2 | w2loc
B_B2 = 0


def _tb_layout(use_b2):
    w2l_of = 2 * V if use_b2 else 0
    return w2l_of, w2l_of + KH * V


def build_bass(rows: int, use_b2: bool):
    """Build the single-core Bass program for a [rows, 2V] batch shard."""
    import concourse.bacc as bacc
    import concourse.bass as bass
    import concourse.tile as tile
    from concourse import mybir

    f32 = mybir.dt.float32
    i32 = mybir.dt.int32
    u32 = mybir.dt.uint32
    A = mybir.AluOpType
    AF = mybir.ActivationFunctionType

    assert rows == P * NJ
    B_W2L, B_W = _tb_layout(use_b2)

    nc = bacc.Bacc(None)
    x = nc.declare_dram_parameter("x", [rows, 2 * V], f32, isOutput=False)
    ta = nc.declare_dram_parameter("ta", [P, A_W], f32, isOutput=False)
    tb = nc.declare_dram_parameter("tb", [P, B_W], f32, isOutput=False)
    out = nc.declare_dram_parameter("out", [rows, 2 * V], f32, isOutput=True)

    x_r = x.rearrange("(p j) n -> p j n", j=NJ)
    out_r = out.rearrange("(p j) n -> p j n", j=NJ)

    def bcast_mid(t_ap, reps):
        return bass.AP(
            tensor=t_ap.tensor, offset=t_ap.offset,
            ap=[t_ap.ap[0], [0, reps]] + list(t_ap.ap[1:]),
        )

    def bcast_last(t_ap, reps):
        return bass.AP(
            tensor=t_ap.tensor, offset=t_ap.offset,
            ap=list(t_ap.ap) + [[0, reps]],
        )

    with tile.TileContext(nc) as tc:
        with (
            tc.tile_pool(name="tab", bufs=1) as tab,
            tc.tile_pool(name="loop", bufs=1) as loop,
            tc.tile_pool(name="ps_net", bufs=2, space="PSUM") as ps_net,
            tc.tile_pool(name="ps_row", bufs=1, space="PSUM") as ps_row,
        ):
            # ---- t0: 3 load DMAs on the SP sequencer, in order of need ----
            ta_sb = tab.tile([P, A_W], f32)
            nc.sync.dma_start(out=ta_sb, in_=ta[:, :])
            tb_sb = tab.tile([P, B_W], f32)
            nc.sync.dma_start(out=tb_sb, in_=tb[:, :])
            xt = loop.tile([P, NJ, 2 * V], f32)
            nc.sync.dma_start(out=xt, in_=x_r[:, :, :])

            w1t_sb = ta_sb[:, A_W1T : A_W1T + KH * V].rearrange("p (k i) -> p k i", k=KH)
            b1_sb = ta_sb[:, A_B1 : A_B1 + KH]
            w2s = ta_sb[:, A_W2S : A_W2S + KH * V].rearrange("p (k c) -> p k c", k=KH)
            w2l = tb_sb[:, B_W2L : B_W2L + KH * V].rearrange("p (k c) -> p k c", k=KH)

            # on-device constants (Pool engine, no input deps)
            iota_i = tab.tile([P, V], i32)
            nc.gpsimd.iota(iota_i, pattern=[[1, V]], base=0, channel_multiplier=0)
            iota_f = tab.tile([P, V], f32)
            nc.vector.tensor_copy(iota_f, iota_i)
            ones_pv = tab.tile([P, V], f32)
            nc.gpsimd.memset(ones_pv, 1.0)
            ident = tab.tile([P, V], f32)
            nc.gpsimd.affine_select(
                out=ident, in_=ones_pv, pattern=[[1, V]],
                compare_op=A.is_equal, fill=0.0, base=0, channel_multiplier=-1,
            )

            # ---- table: NET = relu(W1+b1) @ W2 (+ b2), scale head first ----
            hT = tab.tile([P, KH, V], f32)
            for k in range(KH):
                nc.scalar.activation(
                    hT[:, k, :], w1t_sb[:, k, :], AF.Relu,
                    bias=b1_sb[:, k : k + 1], scale=1.0,
                )
            halves = {}
            for head, w2half in ((1, w2s), (0, w2l)):  # 1=scale first, 0=loc
                ps = ps_net.tile([P, V], f32, tag=f"net{head}")
                for k in range(KH):
                    last = k == KH - 1 and not use_b2
                    nc.tensor.matmul(
                        ps, lhsT=hT[:, k, :], rhs=w2half[:, k],
                        start=(k == 0), stop=last,
                    )
                if use_b2:
                    nc.tensor.matmul(
                        ps, lhsT=ones_pv[0:1, :],
                        rhs=tb_sb[0:1, B_B2 + head * V : B_B2 + (head + 1) * V],
                        start=False, stop=True,
                    )
                halves[head] = ps

            crep_ps = ps_row.tile([P, V], f32)

            with tc.high_priority():
                # scale-head argmax + its half of the pack row
                ixS = tab.tile([P, 8], u32)
                m8S = tab.tile([P, 8], f32)
                nc.vector.max(m8S, halves[1])
                nc.vector.max_index(ixS, m8S, halves[1])
                sfT = tab.tile([P, 1], f32)
                nc.vector.tensor_copy(sfT, ixS[:, 0:1])
                zf = tab.tile([P, 1], f32)
                nc.vector.tensor_scalar(out=zf, in0=sfT, scalar1=0.5, scalar2=None, op0=A.is_le)
                qS = tab.tile([P, 1], f32)
                nc.vector.tensor_scalar(out=qS, in0=sfT, scalar1=32768.0, scalar2=None, op0=A.mult)
                qS2 = tab.tile([P, 1], f32)
                nc.vector.tensor_scalar(out=qS2, in0=zf, scalar1=float(1 << 22), scalar2=qS, op0=A.mult, op1=A.add)
                diagS = tab.tile([P, V], f32)
                nc.vector.tensor_scalar(out=diagS, in0=ident, scalar1=qS2, scalar2=None, op0=A.mult)
                nc.tensor.matmul(crep_ps, lhsT=ones_pv, rhs=diagS, start=True, stop=False)

            # ---- a1 mult (x arrives early now): after the S-chain, before
            # the loc argmax; gates pin the DVE order S-chain -> a1 mult ->
            # maxL/findL -> a1 reduce so nothing stalls on x or NET-loc.
            a1f = tab.tile([P, NJ], f32)
            a1s = loop.tile([P, NJ, V], f32, tag="a1scr")
            nc.vector.tensor_copy(a1s[0:1, 0, 0:1], diagS[0:1, 0:1])
            nc.vector.tensor_mul(a1s, xt[:, :, V :], bcast_mid(iota_f, NJ))

            # loc-head argmax; pack arithmetic on Act (overlaps DVE)
            ixL = tab.tile([P, 8], u32)
            m8L = tab.tile([P, 8], f32)
            nc.vector.tensor_copy(m8L[0:1, 0:1], a1s[0:1, 0, 0:1])
            nc.vector.max(m8L, halves[0])
            nc.vector.max_index(ixL, m8L, halves[0])
            qL256 = tab.tile([P, 1], f32)
            nc.scalar.activation(qL256, ixL[:, 0:1], AF.Copy, scale=256.0)
            diagL = tab.tile([P, V], f32)
            nc.scalar.activation(diagL, ident, AF.Copy, scale=qL256)
            nc.tensor.matmul(crep_ps, lhsT=ones_pv, rhs=diagL, start=False, stop=True)

            nc.vector.tensor_copy(a1f[0:1, 0:1], ixL[0:1, 0:1])
            nc.vector.reduce_sum(a1f, a1s, axis=mybir.AxisListType.X)

            # ---- comb = pk[i0] + a1 per row (batched mult + reduce) ----
            comb_f = tab.tile([P, NJ], f32)
            lks = loop.tile([P, NJ, V], f32, tag="lkscr")
            nc.vector.tensor_mul(lks, xt[:, :, 0:V], bcast_mid(crep_ps, NJ))
            nc.vector.reduce_sum(comb_f, lks, axis=mybir.AxisListType.X)

            # ---- unpack: c = (S*a1 + L) & 127 | 256*[S==0] ----
            # fused: comb + a1 with i32 output (exact integers < 2^23)
            combi = tab.tile([P, NJ], i32)
            nc.vector.tensor_tensor(out=combi, in0=comb_f, in1=a1f, op=A.add)
            a1i = tab.tile([P, NJ], i32)
            nc.vector.tensor_scalar(out=a1i, in0=combi, scalar1=V - 1, scalar2=None, op0=A.bitwise_and)
            li = tab.tile([P, NJ], i32)
            nc.vector.tensor_scalar(out=li, in0=combi, scalar1=8, scalar2=V - 1, op0=A.arith_shift_right, op1=A.bitwise_and)
            si = tab.tile([P, NJ], i32)
            nc.vector.tensor_scalar(out=si, in0=combi, scalar1=15, scalar2=V - 1, op0=A.arith_shift_right, op1=A.bitwise_and)
            kill = tab.tile([P, NJ], i32)
            nc.vector.tensor_scalar(out=kill, in0=combi, scalar1=14, scalar2=2 * V, op0=A.arith_shift_right, op1=A.bitwise_and)
            ti = tab.tile([P, NJ], i32)
            nc.vector.tensor_mul(ti, si, a1i)
            nc.vector.tensor_add(ti, ti, li)
            ci = tab.tile([P, NJ], i32)
            nc.vector.tensor_scalar(out=ci, in0=ti, scalar1=V - 1, scalar2=None, op0=A.bitwise_and)
            nc.vector.tensor_tensor(out=ci, in0=ci, in1=kill, op=A.bitwise_or)

            # ---- z1 in place over the x1 half + fused store per chunk ----
            for ch in range(NCH):
                js = ch * CJ
                nc.vector.tensor_tensor(
                    out=xt[:, js : js + CJ, V :],
                    in0=bcast_mid(iota_i, CJ),
                    in1=bcast_last(ci[:, js : js + CJ], V),
                    op=A.is_equal,
                )
                nc.sync.dma_start(
                    out=out_r[:, js : js + CJ, :], in_=xt[:, js : js + CJ, :]
                )

    nc.finalize()
    return nc


def _host_w2_halves(W2):
    # w2half[p, k*V + c] = W2[k*P + p, half*V + c]; returns (scale, loc)
    w4 = W2.reshape(KH, P, 2, V).transpose(1, 2, 0, 3)  # [p, half, k, c]
    loc = w4[:, 0].reshape(P, KH * V)
    scale = w4[:, 1].reshape(P, KH * V)
    return scale, loc


def _host_ta(W1, b1, W2) -> np.ndarray:
    ta = np.zeros((P, A_W), np.float32)
    # w1t[p, k*V + i] = W1[i, k*P + p] -- pure layout marshalling
    ta[:, A_W1T : A_W1T + KH * V] = (
        W1.T.reshape(KH, P, V).transpose(1, 0, 2).reshape(P, KH * V)
    )
    ta[:, A_B1 : A_B1 + KH] = b1.reshape(KH, P).T
    ta[:, A_W2S : A_W2S + KH * V] = _host_w2_halves(W2)[0]
    return np.ascontiguousarray(ta)


def _host_tb(W2, b2, use_b2) -> np.ndarray:
    B_W2L, B_W = _tb_layout(use_b2)
    tb = np.zeros((P, B_W), np.float32)
    if use_b2:
        tb[:, B_B2 : B_B2 + 2 * V] = b2.reshape(1, 2 * V)
    tb[:, B_W2L : B_W2L + KH * V] = _host_w2_halves(W2)[1]
    return np.ascontiguousarray(tb)


# Test-harness hooks: extra kwargs for run_bass_kernel_spmd (e.g. trace=True)
# and the last BassKernelResults for profiling. Unused when graded.
RUN_KWARGS: dict = {}
LAST_RESULTS = None


def kernel(**inputs) -> np.ndarray:
    global LAST_RESULTS
    from concourse.bass_utils import run_bass_kernel_spmd

    x = np.ascontiguousarray(np.asarray(inputs["inputs"], dtype=np.float32))
    W1 = np.asarray(inputs["W1"], dtype=np.float32)
    b1 = np.asarray(inputs["b1"], dtype=np.float32)
    W2 = np.asarray(inputs["W2"], dtype=np.float32)
    b2 = np.asarray(inputs["b2"], dtype=np.float32)
    use_b2 = bool(np.any(b2 != 0.0))

    tan = _host_ta(W1, b1, W2)
    tbn = _host_tb(W2, b2, use_b2)

    B = x.shape[0]
    rows = B // N_CORES
    nc = build_bass(rows, use_b2)

    shards = np.split(x, N_CORES, axis=0)
    in_maps = [{"x": s, "ta": tan, "tb": tbn} for s in shards]
    res = run_bass_kernel_spmd(nc, in_maps, list(range(N_CORES)), **RUN_KWARGS)
    LAST_RESULTS = res
    return np.concatenate([r["out"] for r in res.results], axis=0)


# revision 34
# speedup vs baseline: 1.0415x; 1.0219x over previous
"""DiscreteBipartiteFlow forward on 8 trn2 NeuronCores.

Math: inputs rows are exact one-hots (x0|x1). net = relu(x0@W1+b1)@W2+b2
only depends on i0=argmax(x0), so precompute (on device, per core) the
[V, 2V] table NET = relu(W1+b1)@W2+b2 and its per-row argmaxes
L[i]=argmax(NET[i,:V]), S[i]=argmax(NET[i,V:]). The straight-through
one_hot_argmax is numerically exactly-hard, one_hot_multiply of one-hots
is an index product, one_hot_add an index sum, so
z1 = one_hot((L[i0] + a1*S[i0]) mod V) (0 when S[i0]==0). Out = [x0|z1].

v10 (per core, 1024 rows, 8 rows per partition). Real-HW profile is
descriptor-count-bound on DMA (~200ns/descriptor/queue for 2-8KB) and
DVE/latency-bound on the dependent chain:
 - 3 load DMAs / 384 descriptors, fused by when they're needed:
   A = w1t|b1|w2-scale-half (4.1KB/partition), B = w2-loc-half (+b2
   when nonzero), C = input (8KB). w2 halves host-permuted to be
   contiguous per partition. iota/ident/ones are generated on the Pool
   engine (iota / affine_select / memset) -- no const DMA.
 - scale head computed FIRST on PE so its argmax+pack chain overlaps
   the loc-head matmuls; b2 matmuls skipped when b2 == 0 (host-checked;
   spec fill is zeros).
 - pack row pk[i] = 256*L[i] + 32768*S[i] + 2^22*[S[i]==0] replicated
   to all partitions via TWO PSUM-accumulated diag matmuls
   (crep = ones @ (ident*packS) + ones @ (ident*packL)); the lookup
   mult reads crep straight from PSUM (no copy). The loc-side pack
   arithmetic runs on the Activation engine (per-partition scale AP);
   WAW gate copies pin the DVE stream order to S-chain -> a1 mult ->
   loc argmax -> a1 reduce so DVE never stalls on x arrival.
 - data side: one batched mult+reduce of the x1 half vs iota (a1), one
   of the x0 half vs crep; comb = pk[i0] + a1, exact in fp32 (< 2^23).
 - int32 unpack (10 ops), z1 built in place over the x1 half via two
   broadcast is_equal chunks, two fused stores (4KB descriptors).
Data-parallel over 8 cores; weights replicated.
"""

import numpy as np

V = 128
H = 512
N_CORES = 8
P = 128
NJ = 8               # row slots per partition
NCH = 2              # store chunks
CJ = NJ // NCH

KH = H // P
# tensor A field offsets (f32 elements per partition): w1t | b1 | w2scale
A_W1T = 0
A_B1 = KH * V
A_W2S = KH * V + KH
A_W = KH * V + KH + KH * V
# tensor B field offsets: iota | ident | ones | b2 | w2loc
B_B2 = 0


def _tb_layout(use_b2):
    w2l_of = 2 * V if use_b2 else 0
    return w2l_of, w2l_of + KH * V


def build_bass(rows: int, use_b2: bool):
    """Build the single-core Bass program for a [rows, 2V] batch shard."""
    import concourse.bacc as bacc
    import concourse.bass as bass
    import concourse.tile as tile
    from concourse import mybir

    f32 = mybir.dt.float32
    i32 = mybir.dt.int32
    u32 = mybir.dt.uint32
    A = mybir.AluOpType
    AF = mybir.ActivationFunctionType

    assert rows == P * NJ
    B_W2L, B_W = _tb_layout(use_b2)

    nc = bacc.Bacc(None)
    x = nc.declare_dram_parameter("x", [rows, 2 * V], f32, isOutput=False)
    ta = nc.declare_dram_parameter("ta", [P, A_W], f32, isOutput=False)
    tb = nc.declare_dram_parameter("tb", [P, B_W], f32, isOutput=False)
    out = nc.declare_dram_parameter("out", [rows, 2 * V], f32, isOutput=True)

    x_r = x.rearrange("(p j) n -> p j n", j=NJ)
    out_r = out.rearrange("(p j) n -> p j n", j=NJ)

    def bcast_mid(t_ap, reps):
        return bass.AP(
            tensor=t_ap.tensor, offset=t_ap.offset,
            ap=[t_ap.ap[0], [0, reps]] + list(t_ap.ap[1:]),
        )

    def bcast_last(t_ap, reps):
        return bass.AP(
            tensor=t_ap.tensor, offset=t_ap.offset,
            ap=list(t_ap.ap) + [[0, reps]],
        )

    with tile.TileContext(nc) as tc:
        with (
            tc.tile_pool(name="tab", bufs=1) as tab,
            tc.tile_pool(name="loop", bufs=1) as loop,
            tc.tile_pool(name="ps_net", bufs=2, space="PSUM") as ps_net,
            tc.tile_pool(name="ps_row", bufs=1, space="PSUM") as ps_row,
        ):
            # ---- t0: 3 load DMAs on the SP sequencer, in order of need ----
            ta_sb = tab.tile([P, A_W], f32)
            nc.sync.dma_start(out=ta_sb, in_=ta[:, :])
            tb_sb = tab.tile([P, B_W], f32)
            nc.sync.dma_start(out=tb_sb, in_=tb[:, :])
            xt = loop.tile([P, NJ, 2 * V], f32)
            nc.sync.dma_start(out=xt, in_=x_r[:, :, :])

            w1t_sb = ta_sb[:, A_W1T : A_W1T + KH * V].rearrange("p (k i) -> p k i", k=KH)
            b1_sb = ta_sb[:, A_B1 : A_B1 + KH]
            w2s = ta_sb[:, A_W2S : A_W2S + KH * V].rearrange("p (k c) -> p k c", k=KH)
            w2l = tb_sb[:, B_W2L : B_W2L + KH * V].rearrange("p (k c) -> p k c", k=KH)

            # on-device constants (Pool engine, no input deps)
            iota_i = tab.tile([P, V], i32)
            nc.gpsimd.iota(iota_i, pattern=[[1, V]], base=0, channel_multiplier=0)
            iota_f = tab.tile([P, V], f32)
            nc.vector.tensor_copy(iota_f, iota_i)
            ones_pv = tab.tile([P, V], f32)
            nc.gpsimd.memset(ones_pv, 1.0)
            ident = tab.tile([P, V], f32)
            nc.gpsimd.affine_select(
                out=ident, in_=ones_pv, pattern=[[1, V]],
                compare_op=A.is_equal, fill=0.0, base=0, channel_multiplier=-1,
            )

            # ---- table: NET = relu(W1+b1) @ W2 (+ b2), scale head first ----
            hT = tab.tile([P, KH, V], f32)
            for k in range(KH):
                nc.scalar.activation(
                    hT[:, k, :], w1t_sb[:, k, :], AF.Relu,
                    bias=b1_sb[:, k : k + 1], scale=1.0,
                )
            halves = {}
            for head, w2half in ((1, w2s), (0, w2l)):  # 1=scale first, 0=loc
                ps = ps_net.tile([P, V], f32, tag=f"net{head}")
                for k in range(KH):
                    last = k == KH - 1 and not use_b2
                    nc.tensor.matmul(
                        ps, lhsT=hT[:, k, :], rhs=w2half[:, k],
                        start=(k == 0), stop=last,
                    )
                if use_b2:
                    nc.tensor.matmul(
                        ps, lhsT=ones_pv[0:1, :],
                        rhs=tb_sb[0:1, B_B2 + head * V : B_B2 + (head + 1) * V],
                        start=False, stop=True,
                    )
                halves[head] = ps

            crep_ps = ps_row.tile([P, V], f32)

            with tc.high_priority():
                # scale-head argmax on DVE; the whole S pack chain runs on
                # the Activation engine (zf = Relu(1 - S) is the S==0 flag)
                ixS = tab.tile([P, 8], u32)
                m8S = tab.tile([P, 8], f32)
                nc.vector.max(m8S, halves[1])
                nc.vector.max_index(ixS, m8S, halves[1])
            qS = tab.tile([P, 1], f32)
            nc.scalar.activation(qS, ixS[:, 0:1], AF.Copy, scale=32768.0)
            zf = tab.tile([P, 1], f32)
            nc.scalar.activation(zf, ixS[:, 0:1], AF.Relu, scale=-1.0, bias=1.0)
            qS2 = tab.tile([P, 1], f32)
            nc.scalar.activation(qS2, zf, AF.Relu, scale=float(1 << 22), bias=qS)
            diagS = tab.tile([P, V], f32)
            nc.scalar.activation(diagS, ident, AF.Copy, scale=qS2)
            nc.tensor.matmul(crep_ps, lhsT=ones_pv, rhs=diagS, start=True, stop=False)

            # ---- a1 mult: right after findS on DVE (gate); S-chain runs
            # concurrently on Act. maxL/findL then a1 reduce follow on DVE.
            a1f = tab.tile([P, NJ], f32)
            a1s = loop.tile([P, NJ, V], f32, tag="a1scr")
            nc.vector.tensor_copy(a1s[0:1, 0, 0:1], ixS[0:1, 0:1])
            nc.vector.tensor_mul(a1s, xt[:, :, V :], bcast_mid(iota_f, NJ))

            # loc-head argmax; pack arithmetic on Act (overlaps DVE)
            ixL = tab.tile([P, 8], u32)
            m8L = tab.tile([P, 8], f32)
            nc.vector.tensor_copy(m8L[0:1, 0:1], a1s[0:1, 0, 0:1])
            nc.vector.max(m8L, halves[0])
            nc.vector.max_index(ixL, m8L, halves[0])
            qL256 = tab.tile([P, 1], f32)
            nc.scalar.activation(qL256, ixL[:, 0:1], AF.Copy, scale=256.0)
            diagL = tab.tile([P, V], f32)
            nc.scalar.activation(diagL, ident, AF.Copy, scale=qL256)
            nc.tensor.matmul(crep_ps, lhsT=ones_pv, rhs=diagL, start=False, stop=True)

            nc.vector.tensor_copy(a1f[0:1, 0:1], ixL[0:1, 0:1])
            nc.vector.reduce_sum(a1f, a1s, axis=mybir.AxisListType.X)

            # ---- comb = pk[i0] + a1 per row (batched mult + reduce) ----
            comb_f = tab.tile([P, NJ], f32)
            lks = loop.tile([P, NJ, V], f32, tag="lkscr")
            nc.vector.tensor_mul(lks, xt[:, :, 0:V], bcast_mid(crep_ps, NJ))
            nc.vector.reduce_sum(comb_f, lks, axis=mybir.AxisListType.X)

            # ---- unpack: c = (S*a1 + L) & 127 | 256*[S==0] ----
            # fused: comb + a1 with i32 output (exact integers < 2^23)
            combi = tab.tile([P, NJ], i32)
            nc.vector.tensor_tensor(out=combi, in0=comb_f, in1=a1f, op=A.add)
            a1i = tab.tile([P, NJ], i32)
            nc.vector.tensor_scalar(out=a1i, in0=combi, scalar1=V - 1, scalar2=None, op0=A.bitwise_and)
            li = tab.tile([P, NJ], i32)
            nc.vector.tensor_scalar(out=li, in0=combi, scalar1=8, scalar2=V - 1, op0=A.arith_shift_right, op1=A.bitwise_and)
            si = tab.tile([P, NJ], i32)
            nc.vector.tensor_scalar(out=si, in0=combi, scalar1=15, scalar2=V - 1, op0=A.arith_shift_right, op1=A.bitwise_and)
            kill = tab.tile([P, NJ], i32)
            nc.vector.tensor_scalar(out=kill, in0=combi, scalar1=14, scalar2=2 * V, op0=A.arith_shift_right, op1=A.bitwise_and)
            ti = tab.tile([P, NJ], i32)
            nc.vector.tensor_mul(ti, si, a1i)
            nc.vector.tensor_add(ti, ti, li)
            ci = tab.tile([P, NJ], i32)
            nc.vector.tensor_scalar(out=ci, in0=ti, scalar1=V - 1, scalar2=None, op0=A.bitwise_and)
            nc.vector.tensor_tensor(out=ci, in0=ci, in1=kill, op=A.bitwise_or)

            # ---- z1 in place over the x1 half + fused store per chunk ----
            for ch in range(NCH):
                js = ch * CJ
                nc.vector.tensor_tensor(
                    out=xt[:, js : js + CJ, V :],
                    in0=bcast_mid(iota_i, CJ),
                    in1=bcast_last(ci[:, js : js + CJ], V),
                    op=A.is_equal,
                )
                nc.sync.dma_start(
                    out=out_r[:, js : js + CJ, :], in_=xt[:, js : js + CJ, :]
                )

    nc.finalize()
    return nc


def _host_w2_halves(W2):
    # w2half[p, k*V + c] = W2[k*P + p, half*V + c]; returns (scale, loc)
    w4 = W2.reshape(KH, P, 2, V).transpose(1, 2, 0, 3)  # [p, half, k, c]
    loc = w4[:, 0].reshape(P, KH * V)
    scale = w4[:, 1].reshape(P, KH * V)
    return scale, loc


def _host_ta(W1, b1, W2) -> np.ndarray:
    ta = np.zeros((P, A_W), np.float32)
    # w1t[p, k*V + i] = W1[i, k*P + p] -- pure layout marshalling
    ta[:, A_W1T : A_W1T + KH * V] = (
        W1.T.reshape(KH, P, V).transpose(1, 0, 2).reshape(P, KH * V)
    )
    ta[:, A_B1 : A_B1 + KH] = b1.reshape(KH, P).T
    ta[:, A_W2S : A_W2S + KH * V] = _host_w2_halves(W2)[0]
    return np.ascontiguousarray(ta)


def _host_tb(W2, b2, use_b2) -> np.ndarray:
    B_W2L, B_W = _tb_layout(use_b2)
    tb = np.zeros((P, B_W), np.float32)
    if use_b2:
        tb[:, B_B2 : B_B2 + 2 * V] = b2.reshape(1, 2 * V)
    tb[:, B_W2L : B_W2L + KH * V] = _host_w2_halves(W2)[1]
    return np.ascontiguousarray(tb)


# Test-harness hooks: extra kwargs for run_bass_kernel_spmd (e.g. trace=True)
# and the last BassKernelResults for profiling. Unused when graded.
RUN_KWARGS: dict = {}
LAST_RESULTS = None


def kernel(**inputs) -> np.ndarray:
    global LAST_RESULTS
    from concourse.bass_utils import run_bass_kernel_spmd

    x = np.ascontiguousarray(np.asarray(inputs["inputs"], dtype=np.float32))
    W1 = np.asarray(inputs["W1"], dtype=np.float32)
    b1 = np.asarray(inputs["b1"], dtype=np.float32)
    W2 = np.asarray(inputs["W2"], dtype=np.float32)
    b2 = np.asarray(inputs["b2"], dtype=np.float32)
    use_b2 = bool(np.any(b2 != 0.0))

    tan = _host_ta(W1, b1, W2)
    tbn = _host_tb(W2, b2, use_b2)

    B = x.shape[0]
    rows = B // N_CORES
    nc = build_bass(rows, use_b2)

    shards = np.split(x, N_CORES, axis=0)
    in_maps = [{"x": s, "ta": tan, "tb": tbn} for s in shards]
    res = run_bass_kernel_spmd(nc, in_maps, list(range(N_CORES)), **RUN_KWARGS)
    LAST_RESULTS = res
    return np.concatenate([r["out"] for r in res.results], axis=0)
